# revision 14
# baseline (speedup 1.0000x reference)
"""TRN2 Bass kernel for nn_BTGINs (2-layer GIN message passing), 8 NeuronCores.

Design (SPMD — one program, per-core data):
- Host relabels nodes into "slots": 8 cores x 100 tiles x 128 slots. A
  two-stage packer (snake-by-degree core assignment, then per-core greedy on
  in-edge-per-quarter 4-vectors) equalizes per-(window, bucket) edge counts
  across cores so the SPMD chunk quota is uniform (4 chunks/cell, ~2.4% pad).
- Aggregation windows are W=128 dst slots (one tile). Messages are gathered
  node-major ([128 msgs/chunk], 256B bf16 rows) with the SWDGE dma_gather
  (int16 idxs -> 4 quarter-major table buckets of 25600 rows), rotated
  across 4 SWDGE queues. Gather descriptor-gen (~1.9ns/row serial on
  GpSimd) is the pacing engine; everything else hides under it.
- One-hot S [128 msgs, 128 dst] built on DVE via a single is_equal per
  window (window-major dloc layout); PE matmul accumulates agg_fm
  [128 feat, 128 dst] over the window's 16 chunks. Padded messages carry
  dloc=128 which matches no iota column (zero contribution).
- MLP/BN in feature-major layout; BN batch stats via a tiny AllReduce of
  per-feature (sum, sumsq); the linear bias before BN cancels and is
  dropped. Layer-2 stats subtract the analytic contribution of the 300
  empty slots per core (their hl1 is a known constant column).
- Layer-1 output tiles are PE-transposed to node-major; the h table is
  AllGathered in 4 quarter slices pipelined with phase-2 so layer-2
  gathers (per bucket = per quarter) can start early. The table layout is
  quarter-major: row = q*8*qsz + core*qsz + off.
"""

import numpy as np
import ml_dtypes

import concourse.bass as bass
import concourse.bacc as bacc
import concourse.mybir as mybir
import concourse.tile as tile
from concourse import bass_utils, library_config

F = 128
P = 128
W = 128          # aggregation window (dst slots)
NCORES = 8
NBUCK = 4
GW = 10          # windows per gather group
BN_EPS = 1e-5
PAD_DLOC = 128.0  # not in [0, W) -> S row all zero

N_FULL = 100000
TPC_FULL = 100   # tiles/core; 100*128*8 = 102400 slots >= 100000


# ----------------------------------------------------------------------------
# host-side prep
# ----------------------------------------------------------------------------

def _pack(src, dst, n, tpc):
    """Three-stage packing: nodes -> (core, window, pos).

    Stage 1: snake over in-degree-sorted nodes -> cores (equal counts,
    equal degree sums). Stage 2: within each core, snake its nodes over
    out-degree into NBUCK quarter groups (the node's source-bucket label;
    windows [q*wpq, (q+1)*wpq) hold group q). Stage 3: per (core,
    quarter-group), greedy assign nodes to that group's windows balancing
    the in-edge-by-source-label 4-vector toward <= 4*P per (window,
    bucket) cell.
    """
    spc = tpc * P
    nwin = tpc
    wpq = nwin // NBUCK
    deg = np.bincount(dst, minlength=n)
    odeg = np.bincount(src, minlength=n)
    order = np.argsort(-deg, kind="stable")

    core_of = np.empty(n, np.int64)
    idx = 0
    blk = 0
    while idx < n:
        cs = range(NCORES) if blk % 2 == 0 else range(NCORES - 1, -1, -1)
        for c in cs:
            if idx >= n:
                break
            core_of[order[idx]] = c
            idx += 1
        blk += 1

    quarter_of = core_of // 2
    d4 = np.zeros((n, NBUCK), np.int64)
    np.add.at(d4, (dst, quarter_of[src]), 1)

    TARGET = 4 * P
    slot_of_node = np.empty(n, np.int64)
    node_of_slot = np.full(NCORES * spc, -1, np.int64)
    for c in range(NCORES):
        nodes = np.where(core_of == c)[0]
        nd = d4[nodes]
        order_c = np.argsort(-nd.sum(1), kind="stable")
        load = np.zeros((nwin, NBUCK), np.int64)
        cnt = np.zeros(nwin, np.int64)
        for i in order_c:
            v4 = nd[i]
            new = load + v4
            pen = np.maximum(new - TARGET, 0).sum(1)
            score = pen * 100000 + new.max(1)
            score[cnt >= W] = 1 << 60
            t = int(np.argmin(score))
            s = c * spc + t * W + cnt[t]
            slot_of_node[nodes[i]] = s
            node_of_slot[s] = nodes[i]
            cnt[t] += 1
            load[t] += v4
    return slot_of_node, node_of_slot


def _prep(x, src, dst, eps1, tpc):
    n = x.shape[0]
    spc = tpc * P
    nslot = NCORES * spc
    nwin = tpc
    qsz = spc // NBUCK          # local quarter size (slots)
    bsz = nslot // NBUCK        # bucket table rows
    assert bsz <= 32767 and nwin % GW == 0
    ngg = nwin // GW

    slot_of_node, node_of_slot = _pack(src, dst, n, tpc)

    sdst = slot_of_node[dst]
    ssrc = slot_of_node[src]

    core = sdst // spc
    w = (sdst % spc) // W
    gg = w // GW
    buck = ssrc // bsz          # global quarter (core pair) of src
    lidx = ssrc % bsz           # row within bucket table
    dloc = sdst % W

    # sort edges by (core, gg, bucket, window) to match the chunk layout
    key = ((core * ngg + gg) * NBUCK + buck) * GW + (w % GW)
    order = np.argsort(key, kind="stable")
    e_key = key[order]
    e_lidx = lidx[order]
    e_dloc = dloc[order]

    nkey = NCORES * ngg * NBUCK * GW
    cnt = np.bincount(e_key, minlength=nkey)
    starts = np.zeros(nkey, np.int64)
    np.cumsum(cnt[:-1], out=starts[1:])
    cntr = cnt.reshape(NCORES, ngg, NBUCK, GW)
    quota = np.ceil(cntr.max(axis=0) / P).astype(np.int64)  # [ngg, NBUCK, GW]

    # chunk slot layout: per gg, bucket-major then window
    cellpos = np.zeros((ngg, NBUCK, GW), np.int64)
    call_info = []   # per gg: [(bucket, chunk_start, n_chunks)]
    gg_c0 = []
    pos = 0
    for g in range(ngg):
        gg_c0.append(pos)
        calls = []
        for b in range(NBUCK):
            c0 = pos
            for wi in range(GW):
                cellpos[g, b, wi] = pos
                pos += quota[g, b, wi]
            if pos > c0:
                calls.append((b, c0, pos - c0))
        call_info.append(calls)
    total_chunks = pos

    # window-major dloc columns + per-window matmul schedule
    nch_w = quota.sum(axis=1)        # [ngg, GW] chunks per window
    maxnch = int(nch_w.max())
    dbase = np.zeros((ngg, GW), np.int64)
    flat = nch_w.reshape(-1)
    np.cumsum(flat[:-1], out=dbase.reshape(-1)[1:])
    sched = [[] for _ in range(nwin)]         # window -> list of chunk cols
    dcol_of_chunk = np.zeros(total_chunks, np.int64)
    for g in range(ngg):
        for wi in range(GW):
            dc = dbase[g, wi]
            for b in range(NBUCK):
                for j in range(quota[g, b, wi]):
                    mcol = cellpos[g, b, wi] + j
                    sched[g * GW + wi].append(int(mcol))
                    dcol_of_chunk[mcol] = dc
                    dc += 1

    # fill idx / dloc arrays
    idx_arr = np.zeros((NCORES, total_chunks * P), np.int64)
    dloc_arr = np.full((NCORES, total_chunks, P), PAD_DLOC, np.float64)
    for c in range(NCORES):
        for g in range(ngg):
            for b in range(NBUCK):
                for wi in range(GW):
                    k = ((c * ngg + g) * NBUCK + b) * GW + wi
                    cc = cnt[k]
                    s = starts[k]
                    base = cellpos[g, b, wi] * P
                    idx_arr[c, base : base + cc] = e_lidx[s : s + cc]
                    dloc_arr[c, cellpos[g, b, wi] : cellpos[g, b, wi] + (cc + P - 1) // P] \
                        .reshape(-1)[:cc] = e_dloc[s : s + cc]

    # dloc SBUF image [P, total_chunks] bf16, window-major columns
    dloc_sb = np.full((NCORES, P, total_chunks), PAD_DLOC, np.float64)
    for c in range(NCORES):
        dloc_sb[c, :, dcol_of_chunk] = dloc_arr[c]
    dloc_sb = dloc_sb.astype(ml_dtypes.bfloat16)

    # idxs SBUF image: per gather call, wrap message list into 16
    # partitions, replicate to 128
    idx_sb = np.zeros((NCORES, P, total_chunks * 8), np.int16)
    for g in range(ngg):
        for (b, c0, nch) in call_info[g]:
            nmsg = nch * P
            for c in range(NCORES):
                lst = idx_arr[c, c0 * P : c0 * P + nmsg]
                w16 = lst.reshape(nmsg // 16, 16).T
                idx_sb[c, :, c0 * 8 : c0 * 8 + nmsg // 16] = np.tile(
                    w16, (8, 1)
                ).astype(np.int16)

    # tables: core-major global layout (matches AllGather placement)
    x_slot = np.zeros((nslot, F), np.float32)
    m = node_of_slot >= 0
    x_slot[m] = x[node_of_slot[m]]
    x_tab = x_slot.astype(ml_dtypes.bfloat16)
    xs = (1.0 + float(eps1)) * x_slot
    x_own = xs.reshape(NCORES, spc, F).transpose(0, 2, 1).astype(ml_dtypes.bfloat16)

    return dict(
        node_of_slot=node_of_slot,
        nslot=nslot,
        spc=spc,
        qsz=qsz,
        bsz=bsz,
        nwin=nwin,
        ngg=ngg,
        call_info=call_info,
        gg_c0=gg_c0,
        total_chunks=total_chunks,
        sched=sched,
        nch_w=nch_w,
        dbase=dbase,
        maxnch=maxnch,
        idx_sb=idx_sb,
        dloc_sb=dloc_sb,
        x_tab=x_tab,
        x_own=x_own,
    )


# ----------------------------------------------------------------------------
# device program
# ----------------------------------------------------------------------------

def _build(tpc, pr, eps2, n_bn, no_collectives=False, core0=0):
    BF = mybir.dt.bfloat16
    FP = mybir.dt.float32
    spc = tpc * P
    nslot = NCORES * spc
    nwin = pr["nwin"]
    ngg = pr["ngg"]
    qsz = pr["qsz"]
    bsz = pr["bsz"]
    call_info = pr["call_info"]
    gg_c0 = pr["gg_c0"]
    sched = pr["sched"]
    nch_w = pr["nch_w"]
    dbase = pr["dbase"]
    maxnch = pr["maxnch"]
    total_chunks = pr["total_chunks"]
    rg = [list(range(NCORES))]
    n_empty = spc - n_bn // NCORES   # empty slots per core (uniform)
    wpq = nwin // NBUCK              # windows per table quarter

    max_gg_chunks = max(sum(nc_ for (_, _, nc_) in call_info[g]) for g in range(ngg))

    nc = bacc.Bacc(
        "TRN2", target_bir_lowering=False, debug=False, num_swdge_queues=4
    )

    x_tab = nc.declare_dram_parameter("x_tab", [nslot, F], BF, isOutput=False)
    idxs = nc.declare_dram_parameter(
        "idxs", [P, total_chunks * 8], mybir.dt.int16, isOutput=False
    )
    dlocs = nc.declare_dram_parameter("dlocs", [P, total_chunks], BF, isOutput=False)
    x_own = nc.declare_dram_parameter("x_own", [P, spc], BF, isOutput=False)
    w1a = nc.declare_dram_parameter("w1a", [F, F], BF, isOutput=False)
    w1b = nc.declare_dram_parameter("w1b", [F, F], BF, isOutput=False)
    w2a = nc.declare_dram_parameter("w2a", [F, F], BF, isOutput=False)
    w2b = nc.declare_dram_parameter("w2b", [F, F], BF, isOutput=False)
    vecs = nc.declare_dram_parameter("vecs", [P, 7], FP, isOutput=False)
    iota = nc.declare_dram_parameter("iota", [P, W], BF, isOutput=False)
    ident = nc.declare_dram_parameter("ident", [P, P], BF, isOutput=False)
    identf = nc.declare_dram_parameter("identf", [P, P], FP, isOutput=False)
    out_ext = nc.declare_dram_parameter("out", [spc, F], FP, isOutput=True)

    h_shard = nc.dram_tensor("h_shard", [spc, F], BF)
    h_tab = nc.dram_tensor("h_tab", [nslot, F], BF, addr_space="Shared")
    bn_in = nc.dram_tensor("bn_in", [P, 2], FP)
    bn_out = nc.dram_tensor("bn_out", [P, 2], FP, addr_space="Shared")

    with tile.TileContext(nc) as tc:
        import contextlib

        with contextlib.ExitStack() as ctx:
            singles = ctx.enter_context(tc.tile_pool(name="singles", bufs=1))
            msgs_p = ctx.enter_context(tc.tile_pool(name="msgs", bufs=2))
            s_p = ctx.enter_context(tc.tile_pool(name="s", bufs=6))
            h0_p = ctx.enter_context(tc.tile_pool(name="h0", bufs=4))
            own_p = ctx.enter_context(tc.tile_pool(name="own", bufs=4))
            sc_p = ctx.enter_context(tc.tile_pool(name="scratch", bufs=3))
            trs_p = ctx.enter_context(tc.tile_pool(name="trs", bufs=4))
            vec_p = ctx.enter_context(tc.tile_pool(name="vec", bufs=2))
            aggp = ctx.enter_context(tc.tile_pool(name="aggp", bufs=2, space="PSUM"))
            mlpp = ctx.enter_context(tc.tile_pool(name="mlpp", bufs=2, space="PSUM"))
            trp = ctx.enter_context(tc.tile_pool(name="trp", bufs=2, space="PSUM"))

            nc.gpsimd.load_library(library_config.mlp)

            sb_idx = singles.tile([P, total_chunks * 8], mybir.dt.int16)
            nc.sync.dma_start(out=sb_idx[:], in_=idxs[:])
            sb_dloc = singles.tile([P, total_chunks], BF)
            nc.sync.dma_start(out=sb_dloc[:], in_=dlocs[:])
            sb_w = {}
            for nm, t in (("w1a", w1a), ("w1b", w1b), ("w2a", w2a), ("w2b", w2b)):
                sb_w[nm] = singles.tile([F, F], BF, tag=f"sb_{nm}", name=f"sb_{nm}")
                nc.sync.dma_start(out=sb_w[nm][:], in_=t[:])
            sb_iota = singles.tile([P, W], BF)
            nc.sync.dma_start(out=sb_iota[:], in_=iota[:])
            sb_ident = singles.tile([P, P], BF)
            nc.sync.dma_start(out=sb_ident[:], in_=ident[:])
            sb_identf = singles.tile([P, P], FP)
            nc.sync.dma_start(out=sb_identf[:], in_=identf[:])
            sb_vecs = singles.tile([P, 7], FP)
            nc.sync.dma_start(out=sb_vecs[:], in_=vecs[:])

            sb_eps = singles.tile([P, 1], FP)
            nc.vector.memset(sb_eps[:], BN_EPS)
            sb_zero = singles.tile([P, 1], FP)
            nc.vector.memset(sb_zero[:], 0.0)
            sb_h1m = singles.tile([P, spc], BF)
            sb_hl1 = singles.tile([P, spc], BF)
            sb_stat = singles.tile([P, 2 * nwin], FP)
            sb_own_e = singles.tile([P, 1], BF)   # (1+eps2)*hl1 of empty slot

            qrot = [0]

            def layer(li):
                tab = x_tab if li == 0 else h_tab
                wa = sb_w["w1a" if li == 0 else "w2a"]
                wb = sb_w["w1b" if li == 0 else "w2b"]
                o = 0 if li == 0 else 3
                g_ap = sb_vecs[:, o : o + 1]
                bt_ap = sb_vecs[:, o + 1 : o + 2]
                bb_ap = sb_vecs[:, o + 2 : o + 3]

                # ---- phase 1: gather + aggregate + first MLP matmul ----
                for g in range(ngg):
                    c0g = gg_c0[g]
                    msgs = msgs_p.tile([P, max_gg_chunks, F], BF, tag="msgs")
                    for (b, c0, nch) in call_info[g]:
                        nmsg = nch * P
                        nc.gpsimd.dma_gather(
                            msgs[:, c0 - c0g : c0 - c0g + nch, :],
                            tab[b * bsz : (b + 1) * bsz, :],
                            sb_idx[:, c0 * 8 : c0 * 8 + nmsg // 16],
                            nmsg,
                            nmsg,
                            F,
                            single_packet=False,
                            queue_num=qrot[0] % 4,
                        )
                        qrot[0] += 1
                    for wi in range(GW):
                        wg = g * GW + wi
                        nch = int(nch_w[g, wi])
                        dc0 = int(dbase[g, wi])
                        S_all = s_p.tile([P, maxnch, W], BF, tag="S")
                        iota_b = bass.AP(
                            tensor=sb_iota[:].tensor,
                            offset=sb_iota[:].offset,
                            ap=[sb_iota[:].ap[0], [0, nch], sb_iota[:].ap[1]],
                        )
                        nc.vector.tensor_tensor(
                            out=S_all[:, :nch, :],
                            in0=sb_dloc[:, dc0 : dc0 + nch].to_broadcast([P, nch, W]),
                            in1=iota_b,
                            op=mybir.AluOpType.is_equal,
                        )
                        agg = aggp.tile([P, W], FP, tag="agg")
                        for k, mcol in enumerate(sched[wg]):
                            nc.tensor.matmul(
                                agg[:],
                                lhsT=msgs[:, mcol - c0g, :],
                                rhs=S_all[:, k, :],
                                start=(k == 0),
                                stop=(k == nch - 1),
                            )
                        own = own_p.tile([P, W], BF, tag="own")
                        if li == 0:
                            nc.sync.dma_start(
                                out=own[:], in_=x_own[:, wg * W : wg * W + W]
                            )
                        else:
                            nc.scalar.activation(
                                out=own[:],
                                in_=sb_hl1[:, wg * W : wg * W + W],
                                func=mybir.ActivationFunctionType.Copy,
                                scale=float(1.0 + eps2),
                            )
                        h0 = h0_p.tile([P, W], BF, tag="h0")
                        nc.vector.tensor_tensor(
                            out=h0[:],
                            in0=agg[:],
                            in1=own[:],
                            op=mybir.AluOpType.add,
                        )
                        h1m = mlpp.tile([P, W], FP, space="PSUM", tag="mlp")
                        nc.tensor.matmul(
                            h1m[:], lhsT=wa[:], rhs=h0[:], start=True, stop=True
                        )
                        nc.scalar.activation(
                            out=sb_h1m[:, wg * W : wg * W + W],
                            in_=h1m[:],
                            func=mybir.ActivationFunctionType.Copy,
                            accum_out=sb_stat[:, 2 * wg : 2 * wg + 1],
                        )
                        sq = sc_p.tile([P, W], BF, tag="sq")
                        nc.scalar.activation(
                            out=sq[:],
                            in_=h1m[:],
                            func=mybir.ActivationFunctionType.Square,
                            accum_out=sb_stat[:, 2 * wg + 1 : 2 * wg + 2],
                        )

                # ---- BN stats ----
                stat2 = vec_p.tile([P, 2], FP, tag="stat2")
                nc.vector.reduce_sum(
                    out=stat2[:],
                    in_=sb_stat[:].rearrange("p (b two) -> p two b", two=2),
                    axis=mybir.AxisListType.X,
                )
                if li == 1:
                    # subtract the analytic contribution of empty slots:
                    # their h1m = w2a^T @ ((1+eps2)*hl1_empty), hl1_empty
                    # constant; n_empty per core, NCORES cores summed in
                    # the AllReduce (each core corrects its own share).
                    h1m_et = mlpp.tile([P, W], FP, space="PSUM", tag="mlp")
                    h1m_e = h1m_et[:, 0:1]
                    nc.tensor.matmul(
                        h1m_e, lhsT=wa[:], rhs=sb_own_e[:], start=True, stop=True
                    )
                    h1m_es = vec_p.tile([P, 1], FP, tag="h1m_es")
                    nc.scalar.activation(
                        out=h1m_es[:], in_=h1m_e,
                        func=mybir.ActivationFunctionType.Copy,
                    )
                    ecor = vec_p.tile([P, 2], FP, tag="ecor")
                    nc.vector.tensor_scalar_mul(
                        ecor[:, 0:1], h1m_es[:], float(n_empty)
                    )
                    sq_e = vec_p.tile([P, 1], FP, tag="sq_e")
                    nc.vector.tensor_tensor(
                        out=sq_e[:], in0=h1m_es[:], in1=h1m_es[:],
                        op=mybir.AluOpType.mult,
                    )
                    nc.vector.tensor_scalar_mul(
                        ecor[:, 1:2], sq_e[:], float(n_empty)
                    )
                    nc.vector.tensor_tensor(
                        out=stat2[:], in0=stat2[:], in1=ecor[:],
                        op=mybir.AluOpType.subtract,
                    )
                nc.sync.dma_start(out=bn_in[:], in_=stat2[:])
                if no_collectives:
                    nc.sync.dma_start(out=bn_out[:], in_=bn_in[:])
                else:
                    nc.gpsimd.collective_compute(
                        "AllReduce",
                        mybir.AluOpType.add,
                        replica_groups=rg,
                        ins=[bn_in.ap().opt()],
                        outs=[bn_out.ap().opt()],
                    )
                sb_bn = vec_p.tile([P, 2], FP, tag="sb_bn")
                nc.sync.dma_start(out=sb_bn[:], in_=bn_out[:])

                mu = vec_p.tile([P, 1], FP, tag="mu")
                nc.vector.tensor_scalar_mul(mu[:], sb_bn[:, 0:1], 1.0 / n_bn)
                var = vec_p.tile([P, 1], FP, tag="var")
                nc.vector.tensor_scalar_mul(var[:], sb_bn[:, 1:2], 1.0 / n_bn)
                mu2 = vec_p.tile([P, 1], FP, tag="mu2")
                nc.vector.tensor_tensor(
                    out=mu2[:], in0=mu[:], in1=mu[:], op=mybir.AluOpType.mult
                )
                nc.vector.tensor_tensor(
                    out=var[:], in0=var[:], in1=mu2[:], op=mybir.AluOpType.subtract
                )
                sd = vec_p.tile([P, 1], FP, tag="sd")
                nc.scalar.activation(
                    out=sd[:], in_=var[:],
                    func=mybir.ActivationFunctionType.Sqrt, bias=sb_eps[:],
                )
                rinv = vec_p.tile([P, 1], FP, tag="rinv")
                nc.vector.reciprocal(rinv[:], sd[:])
                a_ap = vec_p.tile([P, 1], FP, tag="a")
                nc.vector.tensor_tensor(
                    out=a_ap[:], in0=rinv[:], in1=g_ap, op=mybir.AluOpType.mult
                )
                c_ap = vec_p.tile([P, 1], FP, tag="c")
                nc.vector.tensor_tensor(
                    out=c_ap[:], in0=mu[:], in1=a_ap[:], op=mybir.AluOpType.mult
                )
                nc.vector.tensor_tensor(
                    out=c_ap[:], in0=bt_ap, in1=c_ap[:], op=mybir.AluOpType.subtract
                )

                if li == 0:
                    # constant column of an empty slot after layer-1 phase 2,
                    # pre-scaled by (1+eps2): relu((1+eps2)*(w1b^T relu(c1))
                    # + (1+eps2)*b1b)
                    h1n_e = h0_p.tile([P, 1], BF, tag="h1n_e")
                    nc.scalar.activation(
                        out=h1n_e[:], in_=sb_zero[:],
                        func=mybir.ActivationFunctionType.Relu, bias=c_ap[:],
                    )
                    h2_et = mlpp.tile([P, W], FP, space="PSUM", tag="mlp")
                    h2_e = h2_et[:, 0:1]
                    nc.tensor.matmul(
                        h2_e, lhsT=wb[:], rhs=h1n_e[:], start=True, stop=True
                    )
                    nc.scalar.activation(
                        out=sb_own_e[:], in_=h2_e,
                        func=mybir.ActivationFunctionType.Relu,
                        scale=float(1.0 + eps2),
                        bias=sb_vecs[:, 6:7],
                    )

                # ---- phase 2 ----
                for wg in range(nwin):
                    h1n = h0_p.tile([P, W], BF, tag="h1n")
                    nc.scalar.activation(
                        out=h1n[:],
                        in_=sb_h1m[:, wg * W : wg * W + W],
                        func=mybir.ActivationFunctionType.Relu,
                        scale=a_ap[:],
                        bias=c_ap[:],
                    )
                    h2 = mlpp.tile([P, W], FP, space="PSUM", tag="mlp")
                    nc.tensor.matmul(
                        h2[:], lhsT=wb[:], rhs=h1n[:], start=True, stop=True
                    )
                    if li == 0:
                        nc.vector.tensor_scalar(
                            out=sb_hl1[:, wg * W : wg * W + W],
                            in0=h2[:],
                            scalar1=bb_ap,
                            scalar2=0.0,
                            op0=mybir.AluOpType.add,
                            op1=mybir.AluOpType.max,
                        )
                        trp_t = trp.tile([P, P], BF, space="PSUM", tag="trp")
                        nc.tensor.transpose(
                            out=trp_t[:],
                            in_=sb_hl1[:, wg * W : wg * W + W],
                            identity=sb_ident[:],
                        )
                        trs = trs_p.tile([P, P], BF, tag="trs")
                        nc.vector.tensor_scalar_add(trs[:], trp_t[:], 0.0)
                        nc.sync.dma_start(
                            out=h_shard[wg * P : (wg + 1) * P, :], in_=trs[:]
                        )
                    else:
                        of32 = sc_p.tile([P, W], FP, tag="of32")
                        nc.vector.tensor_scalar_add(of32[:], h2[:], bb_ap)
                        trp_t = trp.tile([P, P], FP, space="PSUM", tag="trpf")
                        nc.tensor.transpose(
                            out=trp_t[:],
                            in_=of32[:],
                            identity=sb_identf[:],
                        )
                        trs = trs_p.tile([P, P], FP, tag="trsf")
                        nc.vector.tensor_scalar_add(trs[:], trp_t[:], 0.0)
                        nc.sync.dma_start(
                            out=out_ext[wg * P : (wg + 1) * P, :], in_=trs[:]
                        )

                if li == 0:
                    if no_collectives:
                        nc.sync.dma_start(
                            out=h_tab[core0 * spc : (core0 + 1) * spc, :],
                            in_=h_shard[:],
                        )
                    else:
                        nc.gpsimd.collective_compute(
                            "AllGather",
                            mybir.AluOpType.bypass,
                            replica_groups=rg,
                            ins=[h_shard.ap().opt()],
                            outs=[h_tab.ap().opt()],
                        )

            layer(0)
            layer(1)

    nc.compile()
    return nc


# ----------------------------------------------------------------------------
# entry
# ----------------------------------------------------------------------------

def _make_inputs(pr, inputs, eps2):
    vecs = np.stack(
        [
            np.asarray(inputs["g1"], np.float32),
            np.asarray(inputs["bt1"], np.float32),
            np.asarray(inputs["b1b"], np.float32),
            np.asarray(inputs["g2"], np.float32),
            np.asarray(inputs["bt2"], np.float32),
            np.asarray(inputs["b2b"], np.float32),
            (1.0 + eps2) * np.asarray(inputs["b1b"], np.float32),
        ],
        axis=1,
    )
    bfl = ml_dtypes.bfloat16
    iota = np.tile(np.arange(W, dtype=np.float32), (P, 1)).astype(bfl)
    ident = np.eye(P, dtype=np.float32).astype(bfl)
    identf = np.eye(P, dtype=np.float32)
    w = {
        k: np.asarray(inputs[k], np.float32).astype(bfl)
        for k in ("w1a", "w1b", "w2a", "w2b")
    }
    in_maps = []
    for c in range(NCORES):
        in_maps.append(
            dict(
                x_tab=pr["x_tab"],
                idxs=pr["idx_sb"][c],
                dlocs=pr["dloc_sb"][c],
                x_own=pr["x_own"][c],
                vecs=vecs, iota=iota, ident=ident, identf=identf, **w,
            )
        )
    return in_maps


def _run(inputs, tpc, n_bn, trace=False):
    x = np.asarray(inputs["x"], np.float32)
    src = np.asarray(inputs["src"], np.int64)
    dst = np.asarray(inputs["dst"], np.int64)
    eps1 = float(np.asarray(inputs["eps1"]))
    eps2 = float(np.asarray(inputs["eps2"]))

    pr = _prep(x, src, dst, eps1, tpc)
    nc = _build(tpc, pr, eps2, n_bn)
    in_maps = _make_inputs(pr, inputs, eps2)
    res = bass_utils.run_bass_kernel_spmd(
        nc, in_maps, list(range(NCORES)), trace=trace
    )
    outs = [np.asarray(res.results[c]["out"], np.float32) for c in range(NCORES)]
    out_slot = np.concatenate(outs, axis=0)
    nos = pr["node_of_slot"]
    m = nos >= 0
    out = np.zeros((x.shape[0], F), np.float32)
    out[nos[m]] = out_slot[m]
    if trace:
        return out, res
    return out


def kernel(**inputs) -> np.ndarray:
    return _run(inputs, TPC_FULL, N_FULL)


# revision 19
# speedup vs baseline: 1.1480x; 1.1480x over previous
"""TRN2 Bass kernel for nn_BTGINs (2-layer GIN message passing), 8 NeuronCores.

Design (SPMD — one program, per-core data):
- Host relabels nodes into "slots": 8 cores x 100 tiles x 128 slots. A
  two-stage packer (snake-by-degree core assignment, then per-core greedy on
  in-edge-per-quarter 4-vectors) equalizes per-(window, bucket) edge counts
  across cores so the SPMD chunk quota is uniform (4 chunks/cell, ~2.4% pad).
- Aggregation windows are W=128 dst slots (one tile). Messages are gathered
  node-major ([128 msgs/chunk], 256B bf16 rows) with the SWDGE dma_gather
  (int16 idxs -> 4 quarter-major table buckets of 25600 rows), rotated
  across 4 SWDGE queues. Gather descriptor-gen (~1.9ns/row serial on
  GpSimd) is the pacing engine; everything else hides under it.
- One-hot S [128 msgs, 128 dst] built on DVE via a single is_equal per
  window (window-major dloc layout); PE matmul accumulates agg_fm
  [128 feat, 128 dst] over the window's 16 chunks. Padded messages carry
  dloc=128 which matches no iota column (zero contribution).
- MLP/BN in feature-major layout; BN batch stats via a tiny AllReduce of
  per-feature (sum, sumsq); the linear bias before BN cancels and is
  dropped. Layer-2 stats subtract the analytic contribution of the 300
  empty slots per core (their hl1 is a known constant column).
- Layer-1 output tiles are PE-transposed to node-major; the h table is
  AllGathered in 4 quarter slices pipelined with phase-2 so layer-2
  gathers (per bucket = per quarter) can start early. The table layout is
  quarter-major: row = q*8*qsz + core*qsz + off.
"""

import numpy as np
import ml_dtypes

import concourse.bass as bass
import concourse.bacc as bacc
import concourse.mybir as mybir
import concourse.tile as tile
from concourse import bass_utils, library_config

F = 128
P = 128
W = 128          # aggregation window (dst slots)
NCORES = 8
NBUCK = 4
GW = 10          # windows per gather group
BN_EPS = 1e-5
PAD_DLOC = 128.0  # not in [0, W) -> S row all zero

N_FULL = 100000
TPC_FULL = 100   # tiles/core; 100*128*8 = 102400 slots >= 100000


# ----------------------------------------------------------------------------
# host-side prep
# ----------------------------------------------------------------------------

def _pack(src, dst, n, tpc):
    """Three-stage packing: nodes -> (core, window, pos).

    Stage 1: snake over in-degree-sorted nodes -> cores (equal counts,
    equal degree sums). Stage 2: within each core, snake its nodes over
    out-degree into NBUCK quarter groups (the node's source-bucket label;
    windows [q*wpq, (q+1)*wpq) hold group q). Stage 3: per (core,
    quarter-group), greedy assign nodes to that group's windows balancing
    the in-edge-by-source-label 4-vector toward <= 4*P per (window,
    bucket) cell.
    """
    spc = tpc * P
    nwin = tpc
    wpq = nwin // NBUCK
    deg = np.bincount(dst, minlength=n)
    odeg = np.bincount(src, minlength=n)
    order = np.argsort(-deg, kind="stable")

    core_of = np.empty(n, np.int64)
    idx = 0
    blk = 0
    while idx < n:
        cs = range(NCORES) if blk % 2 == 0 else range(NCORES - 1, -1, -1)
        for c in cs:
            if idx >= n:
                break
            core_of[order[idx]] = c
            idx += 1
        blk += 1

    quarter_of = core_of // 2
    d4 = np.zeros((n, NBUCK), np.int64)
    np.add.at(d4, (dst, quarter_of[src]), 1)

    TARGET = 4 * P
    slot_of_node = np.empty(n, np.int64)
    node_of_slot = np.full(NCORES * spc, -1, np.int64)
    for c in range(NCORES):
        nodes = np.where(core_of == c)[0]
        nd = d4[nodes]
        order_c = np.argsort(-nd.sum(1), kind="stable")
        load = np.zeros((nwin, NBUCK), np.int64)
        cnt = np.zeros(nwin, np.int64)
        for i in order_c:
            v4 = nd[i]
            new = load + v4
            pen = np.maximum(new - TARGET, 0).sum(1)
            score = pen * 100000 + new.max(1)
            score[cnt >= W] = 1 << 60
            t = int(np.argmin(score))
            s = c * spc + t * W + cnt[t]
            slot_of_node[nodes[i]] = s
            node_of_slot[s] = nodes[i]
            cnt[t] += 1
            load[t] += v4
    return slot_of_node, node_of_slot


def _prep(x, src, dst, eps1, tpc):
    n = x.shape[0]
    spc = tpc * P
    nslot = NCORES * spc
    nwin = tpc
    qsz = spc // NBUCK          # local quarter size (slots)
    bsz = nslot // NBUCK        # bucket table rows
    assert bsz <= 32767 and nwin % GW == 0
    ngg = nwin // GW

    slot_of_node, node_of_slot = _pack(src, dst, n, tpc)

    sdst = slot_of_node[dst]
    ssrc = slot_of_node[src]

    core = sdst // spc
    w = (sdst % spc) // W
    gg = w // GW
    buck = ssrc // bsz          # global quarter (core pair) of src
    lidx = ssrc % bsz           # row within bucket table
    dloc = sdst % W

    # sort edges by (core, gg, bucket, window) to match the chunk layout
    key = ((core * ngg + gg) * NBUCK + buck) * GW + (w % GW)
    order = np.argsort(key, kind="stable")
    e_key = key[order]
    e_lidx = lidx[order]
    e_dloc = dloc[order]

    nkey = NCORES * ngg * NBUCK * GW
    cnt = np.bincount(e_key, minlength=nkey)
    starts = np.zeros(nkey, np.int64)
    np.cumsum(cnt[:-1], out=starts[1:])
    cntr = cnt.reshape(NCORES, ngg, NBUCK, GW)
    quota = np.ceil(cntr.max(axis=0) / P).astype(np.int64)  # [ngg, NBUCK, GW]

    # chunk slot layout: per gg, bucket-major then window
    cellpos = np.zeros((ngg, NBUCK, GW), np.int64)
    call_info = []   # per gg: [(bucket, chunk_start, n_chunks)]
    gg_c0 = []
    pos = 0
    for g in range(ngg):
        gg_c0.append(pos)
        calls = []
        for b in range(NBUCK):
            c0 = pos
            for wi in range(GW):
                cellpos[g, b, wi] = pos
                pos += quota[g, b, wi]
            if pos > c0:
                calls.append((b, c0, pos - c0))
        call_info.append(calls)
    total_chunks = pos

    # window-major dloc columns + per-window matmul schedule
    nch_w = quota.sum(axis=1)        # [ngg, GW] chunks per window
    maxnch = int(nch_w.max())
    dbase = np.zeros((ngg, GW), np.int64)
    flat = nch_w.reshape(-1)
    np.cumsum(flat[:-1], out=dbase.reshape(-1)[1:])
    sched = [[] for _ in range(nwin)]         # window -> list of chunk cols
    dcol_of_chunk = np.zeros(total_chunks, np.int64)
    for g in range(ngg):
        for wi in range(GW):
            dc = dbase[g, wi]
            for b in range(NBUCK):
                for j in range(quota[g, b, wi]):
                    mcol = cellpos[g, b, wi] + j
                    sched[g * GW + wi].append(int(mcol))
                    dcol_of_chunk[mcol] = dc
                    dc += 1

    # fill idx / dloc arrays
    idx_arr = np.zeros((NCORES, total_chunks * P), np.int64)
    dloc_arr = np.full((NCORES, total_chunks, P), PAD_DLOC, np.float64)
    for c in range(NCORES):
        for g in range(ngg):
            for b in range(NBUCK):
                for wi in range(GW):
                    k = ((c * ngg + g) * NBUCK + b) * GW + wi
                    cc = cnt[k]
                    s = starts[k]
                    base = cellpos[g, b, wi] * P
                    idx_arr[c, base : base + cc] = e_lidx[s : s + cc]
                    dloc_arr[c, cellpos[g, b, wi] : cellpos[g, b, wi] + (cc + P - 1) // P] \
                        .reshape(-1)[:cc] = e_dloc[s : s + cc]

    # dloc SBUF image [P, total_chunks] bf16, window-major columns
    dloc_sb = np.full((NCORES, P, total_chunks), PAD_DLOC, np.float64)
    for c in range(NCORES):
        dloc_sb[c, :, dcol_of_chunk] = dloc_arr[c]
    dloc_sb = dloc_sb.astype(ml_dtypes.bfloat16)

    # idxs SBUF image: per gather call, wrap message list into 16
    # partitions, replicate to 128
    idx_sb = np.zeros((NCORES, P, total_chunks * 8), np.int16)
    for g in range(ngg):
        for (b, c0, nch) in call_info[g]:
            nmsg = nch * P
            for c in range(NCORES):
                lst = idx_arr[c, c0 * P : c0 * P + nmsg]
                w16 = lst.reshape(nmsg // 16, 16).T
                idx_sb[c, :, c0 * 8 : c0 * 8 + nmsg // 16] = np.tile(
                    w16, (8, 1)
                ).astype(np.int16)

    # tables: core-major global layout (matches AllGather placement)
    x_slot = np.zeros((nslot, F), np.float32)
    m = node_of_slot >= 0
    x_slot[m] = x[node_of_slot[m]]
    x_tab = x_slot.astype(ml_dtypes.bfloat16)
    xs = (1.0 + float(eps1)) * x_slot
    x_own = xs.reshape(NCORES, spc, F).transpose(0, 2, 1).astype(ml_dtypes.bfloat16)

    return dict(
        node_of_slot=node_of_slot,
        nslot=nslot,
        spc=spc,
        qsz=qsz,
        bsz=bsz,
        nwin=nwin,
        ngg=ngg,
        call_info=call_info,
        gg_c0=gg_c0,
        total_chunks=total_chunks,
        sched=sched,
        nch_w=nch_w,
        dbase=dbase,
        maxnch=maxnch,
        idx_sb=idx_sb,
        dloc_sb=dloc_sb,
        x_tab=x_tab,
        x_own=x_own,
    )


# ----------------------------------------------------------------------------
# device program
# ----------------------------------------------------------------------------

def _build(tpc, pr, eps2, n_bn, no_collectives=False, core0=0):
    BF = mybir.dt.bfloat16
    FP = mybir.dt.float32
    spc = tpc * P
    nslot = NCORES * spc
    nwin = pr["nwin"]
    ngg = pr["ngg"]
    qsz = pr["qsz"]
    bsz = pr["bsz"]
    call_info = pr["call_info"]
    gg_c0 = pr["gg_c0"]
    sched = pr["sched"]
    nch_w = pr["nch_w"]
    dbase = pr["dbase"]
    maxnch = pr["maxnch"]
    total_chunks = pr["total_chunks"]
    rg = [list(range(NCORES))]
    n_empty = spc - n_bn // NCORES   # empty slots per core (uniform)
    wpq = nwin // NBUCK              # windows per table quarter

    max_gg_chunks = max(sum(nc_ for (_, _, nc_) in call_info[g]) for g in range(ngg))

    nc = bacc.Bacc(
        "TRN2", target_bir_lowering=False, debug=False, num_swdge_queues=4
    )

    x_tab = nc.declare_dram_parameter("x_tab", [nslot, F], BF, isOutput=False)
    idxs = nc.declare_dram_parameter(
        "idxs", [P, total_chunks * 8], mybir.dt.int16, isOutput=False
    )
    dlocs = nc.declare_dram_parameter("dlocs", [P, total_chunks], BF, isOutput=False)
    x_own = nc.declare_dram_parameter("x_own", [P, spc], BF, isOutput=False)
    w1a = nc.declare_dram_parameter("w1a", [F, F], BF, isOutput=False)
    w1b = nc.declare_dram_parameter("w1b", [F, F], BF, isOutput=False)
    w2a = nc.declare_dram_parameter("w2a", [F, F], BF, isOutput=False)
    w2b = nc.declare_dram_parameter("w2b", [F, F], BF, isOutput=False)
    vecs = nc.declare_dram_parameter("vecs", [P, 7], FP, isOutput=False)
    iota = nc.declare_dram_parameter("iota", [P, W], BF, isOutput=False)
    ident = nc.declare_dram_parameter("ident", [P, P], BF, isOutput=False)
    identf = nc.declare_dram_parameter("identf", [P, P], FP, isOutput=False)
    out_ext = nc.declare_dram_parameter("out", [spc, F], FP, isOutput=True)

    h_shard = nc.dram_tensor("h_shard", [spc, F], BF)
    h_tab = nc.dram_tensor("h_tab", [nslot, F], BF, addr_space="Shared")
    bn_in = nc.dram_tensor("bn_in", [P, 2], FP)
    bn_out = nc.dram_tensor("bn_out", [P, 2], FP, addr_space="Shared")

    with tile.TileContext(nc) as tc:
        import contextlib

        with contextlib.ExitStack() as ctx:
            singles = ctx.enter_context(tc.tile_pool(name="singles", bufs=1))
            msgs_p = ctx.enter_context(tc.tile_pool(name="msgs", bufs=2))
            s_p = ctx.enter_context(tc.tile_pool(name="s", bufs=6))
            h0_p = ctx.enter_context(tc.tile_pool(name="h0", bufs=4))
            own_p = ctx.enter_context(tc.tile_pool(name="own", bufs=4))
            sc_p = ctx.enter_context(tc.tile_pool(name="scratch", bufs=3))
            trs_p = ctx.enter_context(tc.tile_pool(name="trs", bufs=4))
            vec_p = ctx.enter_context(tc.tile_pool(name="vec", bufs=2))
            aggp = ctx.enter_context(tc.tile_pool(name="aggp", bufs=3, space="PSUM"))
            mlpp = ctx.enter_context(tc.tile_pool(name="mlpp", bufs=2, space="PSUM"))
            trp = ctx.enter_context(tc.tile_pool(name="trp", bufs=1, space="PSUM"))

            nc.gpsimd.load_library(library_config.mlp)

            sb_idx = singles.tile([P, total_chunks * 8], mybir.dt.int16)
            nc.sync.dma_start(out=sb_idx[:], in_=idxs[:])
            sb_dloc = singles.tile([P, total_chunks], BF)
            nc.sync.dma_start(out=sb_dloc[:], in_=dlocs[:])
            sb_w = {}
            for nm, t in (("w1a", w1a), ("w1b", w1b), ("w2a", w2a), ("w2b", w2b)):
                sb_w[nm] = singles.tile([F, F], BF, tag=f"sb_{nm}", name=f"sb_{nm}")
                nc.sync.dma_start(out=sb_w[nm][:], in_=t[:])
            sb_iota = singles.tile([P, W], BF)
            nc.sync.dma_start(out=sb_iota[:], in_=iota[:])
            sb_ident = singles.tile([P, P], BF)
            nc.sync.dma_start(out=sb_ident[:], in_=ident[:])
            sb_identf = singles.tile([P, P], FP)
            nc.sync.dma_start(out=sb_identf[:], in_=identf[:])
            sb_vecs = singles.tile([P, 7], FP)
            nc.sync.dma_start(out=sb_vecs[:], in_=vecs[:])

            sb_eps = singles.tile([P, 1], FP)
            nc.vector.memset(sb_eps[:], BN_EPS)
            sb_zero = singles.tile([P, 1], FP)
            nc.vector.memset(sb_zero[:], 0.0)
            sb_h1m = singles.tile([P, spc], BF)
            sb_hl1 = singles.tile([P, spc], BF)
            sb_stat = singles.tile([P, 2 * nwin], FP)
            sb_own_e = singles.tile([P, 1], BF)   # (1+eps2)*hl1 of empty slot

            qrot = [0]

            def layer(li, pre=None):
                tab = x_tab if li == 0 else h_tab
                wa = sb_w["w1a" if li == 0 else "w2a"]
                wb = sb_w["w1b" if li == 0 else "w2b"]
                o = 0 if li == 0 else 3
                g_ap = sb_vecs[:, o : o + 1]
                bt_ap = sb_vecs[:, o + 1 : o + 2]
                bb_ap = sb_vecs[:, o + 2 : o + 3]

                # ---- phase 1: gather + aggregate + first MLP matmul ----
                for g in range(ngg):
                    c0g = gg_c0[g]
                    if g == 0 and pre is not None:
                        msgs = pre
                        for k in range(len(call_info[0])):
                            nc.tensor.wait_ge(prep_sems[k], 16)
                    else:
                        msgs = msgs_p.tile([P, max_gg_chunks, F], BF, tag="msgs")
                        for (b, c0, nch) in call_info[g]:
                            nmsg = nch * P
                            nc.gpsimd.dma_gather(
                                msgs[:, c0 - c0g : c0 - c0g + nch, :],
                                tab[b * bsz : (b + 1) * bsz, :],
                                sb_idx[:, c0 * 8 : c0 * 8 + nmsg // 16],
                                nmsg,
                                nmsg,
                                F,
                                single_packet=False,
                                queue_num=qrot[0] % 4,
                            )
                            qrot[0] += 1
                    for wi in range(GW):
                        wg = g * GW + wi
                        nch = int(nch_w[g, wi])
                        dc0 = int(dbase[g, wi])
                        S_all = s_p.tile([P, maxnch, W], BF, tag="S")
                        iota_b = bass.AP(
                            tensor=sb_iota[:].tensor,
                            offset=sb_iota[:].offset,
                            ap=[sb_iota[:].ap[0], [0, nch], sb_iota[:].ap[1]],
                        )
                        nc.vector.tensor_tensor(
                            out=S_all[:, :nch, :],
                            in0=sb_dloc[:, dc0 : dc0 + nch].to_broadcast([P, nch, W]),
                            in1=iota_b,
                            op=mybir.AluOpType.is_equal,
                        )
                        agg = aggp.tile([P, W], FP, tag="agg")
                        for k, mcol in enumerate(sched[wg]):
                            nc.tensor.matmul(
                                agg[:],
                                lhsT=msgs[:, mcol - c0g, :],
                                rhs=S_all[:, k, :],
                                start=(k == 0),
                                stop=(k == nch - 1),
                            )
                        own = own_p.tile([P, W], BF, tag="own")
                        if li == 0:
                            nc.sync.dma_start(
                                out=own[:], in_=x_own[:, wg * W : wg * W + W]
                            )
                        else:
                            nc.scalar.activation(
                                out=own[:],
                                in_=sb_hl1[:, wg * W : wg * W + W],
                                func=mybir.ActivationFunctionType.Copy,
                                scale=float(1.0 + eps2),
                            )
                        h0 = h0_p.tile([P, W], BF, tag="h0")
                        nc.vector.tensor_tensor(
                            out=h0[:],
                            in0=agg[:],
                            in1=own[:],
                            op=mybir.AluOpType.add,
                        )
                        h1m = mlpp.tile([P, W], FP, space="PSUM", tag="mlp")
                        nc.tensor.matmul(
                            h1m[:], lhsT=wa[:], rhs=h0[:], start=True, stop=True
                        )
                        nc.scalar.activation(
                            out=sb_h1m[:, wg * W : wg * W + W],
                            in_=h1m[:],
                            func=mybir.ActivationFunctionType.Copy,
                            accum_out=sb_stat[:, 2 * wg : 2 * wg + 1],
                        )
                        sq = sc_p.tile([P, W], BF, tag="sq")
                        nc.scalar.activation(
                            out=sq[:],
                            in_=h1m[:],
                            func=mybir.ActivationFunctionType.Square,
                            accum_out=sb_stat[:, 2 * wg + 1 : 2 * wg + 2],
                        )

                # ---- BN stats ----
                stat2 = vec_p.tile([P, 2], FP, tag="stat2")
                nc.vector.reduce_sum(
                    out=stat2[:],
                    in_=sb_stat[:].rearrange("p (b two) -> p two b", two=2),
                    axis=mybir.AxisListType.X,
                )
                if li == 1:
                    # subtract the analytic contribution of empty slots:
                    # their h1m = w2a^T @ ((1+eps2)*hl1_empty), hl1_empty
                    # constant; n_empty per core, NCORES cores summed in
                    # the AllReduce (each core corrects its own share).
                    h1m_et = mlpp.tile([P, W], FP, space="PSUM", tag="mlp")
                    h1m_e = h1m_et[:, 0:1]
                    nc.tensor.matmul(
                        h1m_e, lhsT=wa[:], rhs=sb_own_e[:], start=True, stop=True
                    )
                    h1m_es = vec_p.tile([P, 1], FP, tag="h1m_es")
                    nc.scalar.activation(
                        out=h1m_es[:], in_=h1m_e,
                        func=mybir.ActivationFunctionType.Copy,
                    )
                    ecor = vec_p.tile([P, 2], FP, tag="ecor")
                    nc.vector.tensor_scalar_mul(
                        ecor[:, 0:1], h1m_es[:], float(n_empty)
                    )
                    sq_e = vec_p.tile([P, 1], FP, tag="sq_e")
                    nc.vector.tensor_tensor(
                        out=sq_e[:], in0=h1m_es[:], in1=h1m_es[:],
                        op=mybir.AluOpType.mult,
                    )
                    nc.vector.tensor_scalar_mul(
                        ecor[:, 1:2], sq_e[:], float(n_empty)
                    )
                    nc.vector.tensor_tensor(
                        out=stat2[:], in0=stat2[:], in1=ecor[:],
                        op=mybir.AluOpType.subtract,
                    )
                nc.sync.dma_start(out=bn_in[:], in_=stat2[:])
                if no_collectives:
                    nc.sync.dma_start(out=bn_out[:], in_=bn_in[:])
                else:
                    nc.gpsimd.collective_compute(
                        "AllReduce",
                        mybir.AluOpType.add,
                        replica_groups=rg,
                        ins=[bn_in.ap().opt()],
                        outs=[bn_out.ap().opt()],
                    )
                sb_bn = vec_p.tile([P, 2], FP, tag="sb_bn")
                nc.sync.dma_start(out=sb_bn[:], in_=bn_out[:])

                mu = vec_p.tile([P, 1], FP, tag="mu")
                nc.vector.tensor_scalar_mul(mu[:], sb_bn[:, 0:1], 1.0 / n_bn)
                var = vec_p.tile([P, 1], FP, tag="var")
                nc.vector.tensor_scalar_mul(var[:], sb_bn[:, 1:2], 1.0 / n_bn)
                mu2 = vec_p.tile([P, 1], FP, tag="mu2")
                nc.vector.tensor_tensor(
                    out=mu2[:], in0=mu[:], in1=mu[:], op=mybir.AluOpType.mult
                )
                nc.vector.tensor_tensor(
                    out=var[:], in0=var[:], in1=mu2[:], op=mybir.AluOpType.subtract
                )
                sd = vec_p.tile([P, 1], FP, tag="sd")
                nc.scalar.activation(
                    out=sd[:], in_=var[:],
                    func=mybir.ActivationFunctionType.Sqrt, bias=sb_eps[:],
                )
                rinv = vec_p.tile([P, 1], FP, tag="rinv")
                nc.vector.reciprocal(rinv[:], sd[:])
                a_ap = vec_p.tile([P, 1], FP, tag="a")
                nc.vector.tensor_tensor(
                    out=a_ap[:], in0=rinv[:], in1=g_ap, op=mybir.AluOpType.mult
                )
                c_ap = vec_p.tile([P, 1], FP, tag="c")
                nc.vector.tensor_tensor(
                    out=c_ap[:], in0=mu[:], in1=a_ap[:], op=mybir.AluOpType.mult
                )
                nc.vector.tensor_tensor(
                    out=c_ap[:], in0=bt_ap, in1=c_ap[:], op=mybir.AluOpType.subtract
                )

                if li == 0:
                    # constant column of an empty slot after layer-1 phase 2,
                    # pre-scaled by (1+eps2): relu((1+eps2)*(w1b^T relu(c1))
                    # + (1+eps2)*b1b)
                    h1n_e = h0_p.tile([P, 1], BF, tag="h1n_e")
                    nc.scalar.activation(
                        out=h1n_e[:], in_=sb_zero[:],
                        func=mybir.ActivationFunctionType.Relu, bias=c_ap[:],
                    )
                    h2_et = mlpp.tile([P, W], FP, space="PSUM", tag="mlp")
                    h2_e = h2_et[:, 0:1]
                    nc.tensor.matmul(
                        h2_e, lhsT=wb[:], rhs=h1n_e[:], start=True, stop=True
                    )
                    nc.scalar.activation(
                        out=sb_own_e[:], in_=h2_e,
                        func=mybir.ActivationFunctionType.Relu,
                        scale=float(1.0 + eps2),
                        bias=sb_vecs[:, 6:7],
                    )

                # ---- phase 2 ----
                for wg in range(nwin):
                    h1n = h0_p.tile([P, W], BF, tag="h1n")
                    nc.scalar.activation(
                        out=h1n[:],
                        in_=sb_h1m[:, wg * W : wg * W + W],
                        func=mybir.ActivationFunctionType.Relu,
                        scale=a_ap[:],
                        bias=c_ap[:],
                    )
                    h2 = mlpp.tile([P, W], FP, space="PSUM", tag="mlp")
                    nc.tensor.matmul(
                        h2[:], lhsT=wb[:], rhs=h1n[:], start=True, stop=True
                    )
                    if li == 0:
                        nc.vector.tensor_scalar(
                            out=sb_hl1[:, wg * W : wg * W + W],
                            in0=h2[:],
                            scalar1=bb_ap,
                            scalar2=0.0,
                            op0=mybir.AluOpType.add,
                            op1=mybir.AluOpType.max,
                        )
                        trp_t = trp.tile([P, P], BF, space="PSUM", tag="trp")
                        nc.tensor.transpose(
                            out=trp_t[:],
                            in_=sb_hl1[:, wg * W : wg * W + W],
                            identity=sb_ident[:],
                        )
                        trs = trs_p.tile([P, P], BF, tag="trs")
                        nc.vector.tensor_scalar_add(trs[:], trp_t[:], 0.0)
                        nc.sync.dma_start(
                            out=h_shard[wg * P : (wg + 1) * P, :], in_=trs[:]
                        )
                    else:
                        of32 = sc_p.tile([P, W], FP, tag="of32")
                        nc.vector.tensor_scalar_add(of32[:], h2[:], bb_ap)
                        trp_t = trp.tile([P, P], FP, space="PSUM", tag="trpf")
                        nc.tensor.transpose(
                            out=trp_t[:],
                            in_=of32[:],
                            identity=sb_identf[:],
                        )
                        trs = trs_p.tile([P, P], FP, tag="trsf")
                        nc.vector.tensor_scalar_add(trs[:], trp_t[:], 0.0)
                        nc.sync.dma_start(
                            out=out_ext[wg * P : (wg + 1) * P, :], in_=trs[:]
                        )

                if li == 0:
                    if no_collectives:
                        nc.sync.dma_start(
                            out=h_tab[core0 * spc : (core0 + 1) * spc, :],
                            in_=h_shard[:],
                        )
                    else:
                        nc.gpsimd.collective_compute(
                            "AllGather",
                            mybir.AluOpType.bypass,
                            replica_groups=rg,
                            ins=[h_shard.ap().opt()],
                            outs=[h_tab.ap().opt()],
                        )
                    # prep layer-2 gg0 gather descriptors now: desc-gen
                    # overlaps the AllGather's data movement; the DMA fires
                    # via trigger_dma once the table lands.
                    prep_ret[0] = msgs_p.tile(
                        [P, max_gg_chunks, F], BF, tag="msgs", name="prep_msgs"
                    )
                    for k, (b, c0, nch) in enumerate(call_info[0]):
                        nmsg = nch * P
                        nc.gpsimd.dma_gather(
                            prep_ret[0][:, c0 : c0 + nch, :],
                            h_tab[b * bsz : (b + 1) * bsz, :],
                            sb_idx[:, c0 * 8 : c0 * 8 + nmsg // 16],
                            nmsg,
                            nmsg,
                            F,
                            single_packet=False,
                            prepare_only=True,
                            sem=prep_sems[k],
                            queue_num=k,
                        )
                    for k in range(len(call_info[0])):
                        nc.gpsimd.trigger_dma(count=None, queue_num=k)

            prep_ret = [None]
            prep_sems = [
                nc.alloc_semaphore(f"prep_dma_{k}")
                for k in range(len(call_info[0]))
            ]
            layer(0)
            layer(1, pre=prep_ret[0])

    nc.compile()
    return nc


# ----------------------------------------------------------------------------
# entry
# ----------------------------------------------------------------------------

def _make_inputs(pr, inputs, eps2):
    vecs = np.stack(
        [
            np.asarray(inputs["g1"], np.float32),
            np.asarray(inputs["bt1"], np.float32),
            np.asarray(inputs["b1b"], np.float32),
            np.asarray(inputs["g2"], np.float32),
            np.asarray(inputs["bt2"], np.float32),
            np.asarray(inputs["b2b"], np.float32),
            (1.0 + eps2) * np.asarray(inputs["b1b"], np.float32),
        ],
        axis=1,
    )
    bfl = ml_dtypes.bfloat16
    iota = np.tile(np.arange(W, dtype=np.float32), (P, 1)).astype(bfl)
    ident = np.eye(P, dtype=np.float32).astype(bfl)
    identf = np.eye(P, dtype=np.float32)
    w = {
        k: np.asarray(inputs[k], np.float32).astype(bfl)
        for k in ("w1a", "w1b", "w2a", "w2b")
    }
    in_maps = []
    for c in range(NCORES):
        in_maps.append(
            dict(
                x_tab=pr["x_tab"],
                idxs=pr["idx_sb"][c],
                dlocs=pr["dloc_sb"][c],
                x_own=pr["x_own"][c],
                vecs=vecs, iota=iota, ident=ident, identf=identf, **w,
            )
        )
    return in_maps


def _run(inputs, tpc, n_bn, trace=False):
    x = np.asarray(inputs["x"], np.float32)
    src = np.asarray(inputs["src"], np.int64)
    dst = np.asarray(inputs["dst"], np.int64)
    eps1 = float(np.asarray(inputs["eps1"]))
    eps2 = float(np.asarray(inputs["eps2"]))

    pr = _prep(x, src, dst, eps1, tpc)
    nc = _build(tpc, pr, eps2, n_bn)
    in_maps = _make_inputs(pr, inputs, eps2)
    res = bass_utils.run_bass_kernel_spmd(
        nc, in_maps, list(range(NCORES)), trace=trace
    )
    outs = [np.asarray(res.results[c]["out"], np.float32) for c in range(NCORES)]
    out_slot = np.concatenate(outs, axis=0)
    nos = pr["node_of_slot"]
    m = nos >= 0
    out = np.zeros((x.shape[0], F), np.float32)
    out[nos[m]] = out_slot[m]
    if trace:
        return out, res
    return out


def kernel(**inputs) -> np.ndarray:
    return _run(inputs, TPC_FULL, N_FULL)


# revision 23
# speedup vs baseline: 1.1502x; 1.0019x over previous
"""TRN2 Bass kernel for nn_BTGINs (2-layer GIN message passing), 8 NeuronCores.

Design (SPMD — one program, per-core data):
- Host relabels nodes into "slots": 8 cores x 100 tiles x 128 slots. A
  two-stage packer (snake-by-degree core assignment, then per-core greedy on
  in-edge-per-quarter 4-vectors) equalizes per-(window, bucket) edge counts
  across cores so the SPMD chunk quota is uniform (4 chunks/cell, ~2.4% pad).
- Aggregation windows are W=128 dst slots (one tile). Messages are gathered
  node-major ([128 msgs/chunk], 256B bf16 rows) with the SWDGE dma_gather
  (int16 idxs -> 4 quarter-major table buckets of 25600 rows), rotated
  across 4 SWDGE queues. Gather descriptor-gen (~1.9ns/row serial on
  GpSimd) is the pacing engine; everything else hides under it.
- One-hot S [128 msgs, 128 dst] built on DVE via a single is_equal per
  window (window-major dloc layout); PE matmul accumulates agg_fm
  [128 feat, 128 dst] over the window's 16 chunks. Padded messages carry
  dloc=128 which matches no iota column (zero contribution).
- MLP/BN in feature-major layout; BN batch stats via a tiny AllReduce of
  per-feature (sum, sumsq); the linear bias before BN cancels and is
  dropped. Layer-2 stats subtract the analytic contribution of the 300
  empty slots per core (their hl1 is a known constant column).
- Layer-1 output tiles are PE-transposed to node-major; the h table is
  AllGathered in 4 quarter slices pipelined with phase-2 so layer-2
  gathers (per bucket = per quarter) can start early. The table layout is
  quarter-major: row = q*8*qsz + core*qsz + off.
"""

import numpy as np
import ml_dtypes

import concourse.bass as bass
import concourse.bacc as bacc
import concourse.mybir as mybir
import concourse.tile as tile
from concourse import bass_utils, library_config

F = 128
P = 128
W = 128          # aggregation window (dst slots)
NCORES = 8
NBUCK = 4
GW = 10          # windows per gather group
BN_EPS = 1e-5
PAD_DLOC = 128.0  # not in [0, W) -> S row all zero

N_FULL = 100000
TPC_FULL = 100   # tiles/core; 100*128*8 = 102400 slots >= 100000


# ----------------------------------------------------------------------------
# host-side prep
# ----------------------------------------------------------------------------

def _pack(src, dst, n, tpc):
    """Three-stage packing: nodes -> (core, window, pos).

    Stage 1: snake over in-degree-sorted nodes -> cores (equal counts,
    equal degree sums). Stage 2: within each core, snake its nodes over
    out-degree into NBUCK quarter groups (the node's source-bucket label;
    windows [q*wpq, (q+1)*wpq) hold group q). Stage 3: per (core,
    quarter-group), greedy assign nodes to that group's windows balancing
    the in-edge-by-source-label 4-vector toward <= 4*P per (window,
    bucket) cell.
    """
    spc = tpc * P
    nwin = tpc
    wpq = nwin // NBUCK
    deg = np.bincount(dst, minlength=n)
    odeg = np.bincount(src, minlength=n)
    order = np.argsort(-deg, kind="stable")

    core_of = np.empty(n, np.int64)
    idx = 0
    blk = 0
    while idx < n:
        cs = range(NCORES) if blk % 2 == 0 else range(NCORES - 1, -1, -1)
        for c in cs:
            if idx >= n:
                break
            core_of[order[idx]] = c
            idx += 1
        blk += 1

    quarter_of = core_of // 2
    d4 = np.zeros((n, NBUCK), np.int64)
    np.add.at(d4, (dst, quarter_of[src]), 1)

    TARGET = 4 * P
    slot_of_node = np.empty(n, np.int64)
    node_of_slot = np.full(NCORES * spc, -1, np.int64)
    for c in range(NCORES):
        nodes = np.where(core_of == c)[0]
        nd = d4[nodes]
        order_c = np.argsort(-nd.sum(1), kind="stable")
        load = np.zeros((nwin, NBUCK), np.int64)
        cnt = np.zeros(nwin, np.int64)
        for i in order_c:
            v4 = nd[i]
            new = load + v4
            pen = np.maximum(new - TARGET, 0).sum(1)
            score = pen * 100000 + new.max(1)
            score[cnt >= W] = 1 << 60
            t = int(np.argmin(score))
            s = c * spc + t * W + cnt[t]
            slot_of_node[nodes[i]] = s
            node_of_slot[s] = nodes[i]
            cnt[t] += 1
            load[t] += v4
    return slot_of_node, node_of_slot


def _prep(x, src, dst, eps1, tpc):
    n = x.shape[0]
    spc = tpc * P
    nslot = NCORES * spc
    nwin = tpc
    qsz = spc // NBUCK          # local quarter size (slots)
    bsz = nslot // NBUCK        # bucket table rows
    assert bsz <= 32767 and nwin % GW == 0
    ngg = nwin // GW

    slot_of_node, node_of_slot = _pack(src, dst, n, tpc)

    sdst = slot_of_node[dst]
    ssrc = slot_of_node[src]

    core = sdst // spc
    w = (sdst % spc) // W
    gg = w // GW
    buck = ssrc // bsz          # global quarter (core pair) of src
    lidx = ssrc % bsz           # row within bucket table
    dloc = sdst % W

    # sort edges by (core, gg, bucket, window) to match the chunk layout
    key = ((core * ngg + gg) * NBUCK + buck) * GW + (w % GW)
    order = np.argsort(key, kind="stable")
    e_key = key[order]
    e_lidx = lidx[order]
    e_dloc = dloc[order]

    nkey = NCORES * ngg * NBUCK * GW
    cnt = np.bincount(e_key, minlength=nkey)
    starts = np.zeros(nkey, np.int64)
    np.cumsum(cnt[:-1], out=starts[1:])
    cntr = cnt.reshape(NCORES, ngg, NBUCK, GW)
    quota = np.ceil(cntr.max(axis=0) / P).astype(np.int64)  # [ngg, NBUCK, GW]

    # chunk slot layout: per gg, bucket-major then window
    cellpos = np.zeros((ngg, NBUCK, GW), np.int64)
    call_info = []   # per gg: [(bucket, chunk_start, n_chunks)]
    gg_c0 = []
    pos = 0
    for g in range(ngg):
        gg_c0.append(pos)
        calls = []
        for b in range(NBUCK):
            c0 = pos
            for wi in range(GW):
                cellpos[g, b, wi] = pos
                pos += quota[g, b, wi]
            if pos > c0:
                calls.append((b, c0, pos - c0))
        call_info.append(calls)
    total_chunks = pos

    # window-major dloc columns + per-window matmul schedule
    nch_w = quota.sum(axis=1)        # [ngg, GW] chunks per window
    maxnch = int(nch_w.max())
    dbase = np.zeros((ngg, GW), np.int64)
    flat = nch_w.reshape(-1)
    np.cumsum(flat[:-1], out=dbase.reshape(-1)[1:])
    sched = [[] for _ in range(nwin)]         # window -> list of chunk cols
    dcol_of_chunk = np.zeros(total_chunks, np.int64)
    for g in range(ngg):
        for wi in range(GW):
            dc = dbase[g, wi]
            for b in range(NBUCK):
                for j in range(quota[g, b, wi]):
                    mcol = cellpos[g, b, wi] + j
                    sched[g * GW + wi].append(int(mcol))
                    dcol_of_chunk[mcol] = dc
                    dc += 1

    # fill idx / dloc arrays
    idx_arr = np.zeros((NCORES, total_chunks * P), np.int64)
    dloc_arr = np.full((NCORES, total_chunks, P), PAD_DLOC, np.float64)
    for c in range(NCORES):
        for g in range(ngg):
            for b in range(NBUCK):
                for wi in range(GW):
                    k = ((c * ngg + g) * NBUCK + b) * GW + wi
                    cc = cnt[k]
                    s = starts[k]
                    base = cellpos[g, b, wi] * P
                    idx_arr[c, base : base + cc] = e_lidx[s : s + cc]
                    dloc_arr[c, cellpos[g, b, wi] : cellpos[g, b, wi] + (cc + P - 1) // P] \
                        .reshape(-1)[:cc] = e_dloc[s : s + cc]

    # dloc SBUF image [P, total_chunks] bf16, window-major columns
    dloc_sb = np.full((NCORES, P, total_chunks), PAD_DLOC, np.float64)
    for c in range(NCORES):
        dloc_sb[c, :, dcol_of_chunk] = dloc_arr[c]
    dloc_sb = dloc_sb.astype(ml_dtypes.bfloat16)

    # idxs SBUF image: per gather call, wrap message list into 16
    # partitions, replicate to 128
    idx_sb = np.zeros((NCORES, P, total_chunks * 8), np.int16)
    for g in range(ngg):
        for (b, c0, nch) in call_info[g]:
            nmsg = nch * P
            for c in range(NCORES):
                lst = idx_arr[c, c0 * P : c0 * P + nmsg]
                w16 = lst.reshape(nmsg // 16, 16).T
                idx_sb[c, :, c0 * 8 : c0 * 8 + nmsg // 16] = np.tile(
                    w16, (8, 1)
                ).astype(np.int16)

    # tables: core-major global layout (matches AllGather placement)
    x_slot = np.zeros((nslot, F), np.float32)
    m = node_of_slot >= 0
    x_slot[m] = x[node_of_slot[m]]
    x_tab = x_slot.astype(ml_dtypes.bfloat16)
    xs = (1.0 + float(eps1)) * x_slot
    x_own = xs.reshape(NCORES, spc, F).transpose(0, 2, 1).astype(ml_dtypes.bfloat16)

    return dict(
        node_of_slot=node_of_slot,
        nslot=nslot,
        spc=spc,
        qsz=qsz,
        bsz=bsz,
        nwin=nwin,
        ngg=ngg,
        call_info=call_info,
        gg_c0=gg_c0,
        total_chunks=total_chunks,
        sched=sched,
        nch_w=nch_w,
        dbase=dbase,
        maxnch=maxnch,
        idx_sb=idx_sb,
        dloc_sb=dloc_sb,
        x_tab=x_tab,
        x_own=x_own,
    )


# ----------------------------------------------------------------------------
# device program
# ----------------------------------------------------------------------------

def _build(tpc, pr, eps2, n_bn, no_collectives=False, core0=0):
    BF = mybir.dt.bfloat16
    FP = mybir.dt.float32
    spc = tpc * P
    nslot = NCORES * spc
    nwin = pr["nwin"]
    ngg = pr["ngg"]
    qsz = pr["qsz"]
    bsz = pr["bsz"]
    call_info = pr["call_info"]
    gg_c0 = pr["gg_c0"]
    sched = pr["sched"]
    nch_w = pr["nch_w"]
    dbase = pr["dbase"]
    maxnch = pr["maxnch"]
    total_chunks = pr["total_chunks"]
    rg = [list(range(NCORES))]
    n_empty = spc - n_bn // NCORES   # empty slots per core (uniform)
    wpq = nwin // NBUCK              # windows per table quarter

    max_gg_chunks = max(sum(nc_ for (_, _, nc_) in call_info[g]) for g in range(ngg))

    nc = bacc.Bacc(
        "TRN2", target_bir_lowering=False, debug=False, num_swdge_queues=4
    )

    x_tab = nc.declare_dram_parameter("x_tab", [nslot, F], BF, isOutput=False)
    idxs = nc.declare_dram_parameter(
        "idxs", [P, total_chunks * 8], mybir.dt.int16, isOutput=False
    )
    dlocs = nc.declare_dram_parameter("dlocs", [P, total_chunks], BF, isOutput=False)
    x_own = nc.declare_dram_parameter("x_own", [P, spc], BF, isOutput=False)
    w1a = nc.declare_dram_parameter("w1a", [F, F], BF, isOutput=False)
    w1b = nc.declare_dram_parameter("w1b", [F, F], BF, isOutput=False)
    w2a = nc.declare_dram_parameter("w2a", [F, F], BF, isOutput=False)
    w2b = nc.declare_dram_parameter("w2b", [F, F], BF, isOutput=False)
    vecs = nc.declare_dram_parameter("vecs", [P, 7], FP, isOutput=False)
    iota = nc.declare_dram_parameter("iota", [P, W], BF, isOutput=False)
    ident = nc.declare_dram_parameter("ident", [P, P], BF, isOutput=False)
    out_ext = nc.declare_dram_parameter("out", [P, spc], FP, isOutput=True)

    h_shard = nc.dram_tensor("h_shard", [spc, F], BF)
    h_tab = nc.dram_tensor("h_tab", [nslot, F], BF, addr_space="Shared")
    bn_in = nc.dram_tensor("bn_in", [P, 2], FP)
    bn_out = nc.dram_tensor("bn_out", [P, 2], FP, addr_space="Shared")

    with tile.TileContext(nc) as tc:
        import contextlib

        with contextlib.ExitStack() as ctx:
            singles = ctx.enter_context(tc.tile_pool(name="singles", bufs=1))
            msgs_p = ctx.enter_context(tc.tile_pool(name="msgs", bufs=2))
            s_p = ctx.enter_context(tc.tile_pool(name="s", bufs=6))
            h0_p = ctx.enter_context(tc.tile_pool(name="h0", bufs=4))
            own_p = ctx.enter_context(tc.tile_pool(name="own", bufs=4))
            sc_p = ctx.enter_context(tc.tile_pool(name="scratch", bufs=3))
            trs_p = ctx.enter_context(tc.tile_pool(name="trs", bufs=4))
            vec_p = ctx.enter_context(tc.tile_pool(name="vec", bufs=2))
            aggp = ctx.enter_context(tc.tile_pool(name="aggp", bufs=3, space="PSUM"))
            mlpp = ctx.enter_context(tc.tile_pool(name="mlpp", bufs=2, space="PSUM"))
            trp = ctx.enter_context(tc.tile_pool(name="trp", bufs=1, space="PSUM"))

            nc.gpsimd.load_library(library_config.mlp)

            sb_idx = singles.tile([P, total_chunks * 8], mybir.dt.int16)
            nc.sync.dma_start(out=sb_idx[:], in_=idxs[:])
            sb_dloc = singles.tile([P, total_chunks], BF)
            nc.sync.dma_start(out=sb_dloc[:], in_=dlocs[:])
            sb_w = {}
            for nm, t in (("w1a", w1a), ("w1b", w1b), ("w2a", w2a), ("w2b", w2b)):
                sb_w[nm] = singles.tile([F, F], BF, tag=f"sb_{nm}", name=f"sb_{nm}")
                nc.sync.dma_start(out=sb_w[nm][:], in_=t[:])
            sb_iota = singles.tile([P, W], BF)
            nc.sync.dma_start(out=sb_iota[:], in_=iota[:])
            sb_ident = singles.tile([P, P], BF)
            nc.sync.dma_start(out=sb_ident[:], in_=ident[:])
            sb_vecs = singles.tile([P, 7], FP)
            nc.sync.dma_start(out=sb_vecs[:], in_=vecs[:])

            sb_eps = singles.tile([P, 1], FP)
            nc.vector.memset(sb_eps[:], BN_EPS)
            sb_zero = singles.tile([P, 1], FP)
            nc.vector.memset(sb_zero[:], 0.0)
            sb_h1m = singles.tile([P, spc], BF)
            sb_hl1 = singles.tile([P, spc], BF)
            sb_stat = singles.tile([P, 2 * nwin], FP)
            sb_own_e = singles.tile([P, 1], BF)   # (1+eps2)*hl1 of empty slot

            qrot = [0]

            def layer(li):
                tab = x_tab if li == 0 else h_tab
                wa = sb_w["w1a" if li == 0 else "w2a"]
                wb = sb_w["w1b" if li == 0 else "w2b"]
                o = 0 if li == 0 else 3
                g_ap = sb_vecs[:, o : o + 1]
                bt_ap = sb_vecs[:, o + 1 : o + 2]
                bb_ap = sb_vecs[:, o + 2 : o + 3]

                # ---- phase 1: gather + aggregate + first MLP matmul ----
                for g in range(ngg):
                    c0g = gg_c0[g]
                    msgs = msgs_p.tile([P, max_gg_chunks, F], BF, tag="msgs")
                    for (b, c0, nch) in call_info[g]:
                        nmsg = nch * P
                        nc.gpsimd.dma_gather(
                            msgs[:, c0 - c0g : c0 - c0g + nch, :],
                            tab[b * bsz : (b + 1) * bsz, :],
                            sb_idx[:, c0 * 8 : c0 * 8 + nmsg // 16],
                            nmsg,
                            nmsg,
                            F,
                            single_packet=False,
                            queue_num=qrot[0] % 4,
                        )
                        qrot[0] += 1
                    for wi in range(GW):
                        wg = g * GW + wi
                        nch = int(nch_w[g, wi])
                        dc0 = int(dbase[g, wi])
                        S_all = s_p.tile([P, maxnch, W], BF, tag="S")
                        iota_b = bass.AP(
                            tensor=sb_iota[:].tensor,
                            offset=sb_iota[:].offset,
                            ap=[sb_iota[:].ap[0], [0, nch], sb_iota[:].ap[1]],
                        )
                        nc.vector.tensor_tensor(
                            out=S_all[:, :nch, :],
                            in0=sb_dloc[:, dc0 : dc0 + nch].to_broadcast([P, nch, W]),
                            in1=iota_b,
                            op=mybir.AluOpType.is_equal,
                        )
                        agg = aggp.tile([P, W], FP, tag="agg")
                        for k, mcol in enumerate(sched[wg]):
                            nc.tensor.matmul(
                                agg[:],
                                lhsT=msgs[:, mcol - c0g, :],
                                rhs=S_all[:, k, :],
                                start=(k == 0),
                                stop=(k == nch - 1),
                            )
                        own = own_p.tile([P, W], BF, tag="own")
                        if li == 0:
                            nc.sync.dma_start(
                                out=own[:], in_=x_own[:, wg * W : wg * W + W]
                            )
                        else:
                            nc.scalar.activation(
                                out=own[:],
                                in_=sb_hl1[:, wg * W : wg * W + W],
                                func=mybir.ActivationFunctionType.Copy,
                                scale=float(1.0 + eps2),
                            )
                        h0 = h0_p.tile([P, W], BF, tag="h0")
                        nc.vector.tensor_tensor(
                            out=h0[:],
                            in0=agg[:],
                            in1=own[:],
                            op=mybir.AluOpType.add,
                        )
                        h1m = mlpp.tile([P, W], FP, space="PSUM", tag="mlp")
                        nc.tensor.matmul(
                            h1m[:], lhsT=wa[:], rhs=h0[:], start=True, stop=True
                        )
                        nc.scalar.activation(
                            out=sb_h1m[:, wg * W : wg * W + W],
                            in_=h1m[:],
                            func=mybir.ActivationFunctionType.Copy,
                            accum_out=sb_stat[:, 2 * wg : 2 * wg + 1],
                        )
                        sq = sc_p.tile([P, W], BF, tag="sq")
                        nc.scalar.activation(
                            out=sq[:],
                            in_=h1m[:],
                            func=mybir.ActivationFunctionType.Square,
                            accum_out=sb_stat[:, 2 * wg + 1 : 2 * wg + 2],
                        )

                # ---- BN stats ----
                stat2 = vec_p.tile([P, 2], FP, tag="stat2")
                nc.vector.reduce_sum(
                    out=stat2[:],
                    in_=sb_stat[:].rearrange("p (b two) -> p two b", two=2),
                    axis=mybir.AxisListType.X,
                )
                if li == 1:
                    # subtract the analytic contribution of empty slots:
                    # their h1m = w2a^T @ ((1+eps2)*hl1_empty), hl1_empty
                    # constant; n_empty per core, NCORES cores summed in
                    # the AllReduce (each core corrects its own share).
                    h1m_et = mlpp.tile([P, W], FP, space="PSUM", tag="mlp")
                    h1m_e = h1m_et[:, 0:1]
                    nc.tensor.matmul(
                        h1m_e, lhsT=wa[:], rhs=sb_own_e[:], start=True, stop=True
                    )
                    h1m_es = vec_p.tile([P, 1], FP, tag="h1m_es")
                    nc.scalar.activation(
                        out=h1m_es[:], in_=h1m_e,
                        func=mybir.ActivationFunctionType.Copy,
                    )
                    ecor = vec_p.tile([P, 2], FP, tag="ecor")
                    nc.vector.tensor_scalar_mul(
                        ecor[:, 0:1], h1m_es[:], float(n_empty)
                    )
                    sq_e = vec_p.tile([P, 1], FP, tag="sq_e")
                    nc.vector.tensor_tensor(
                        out=sq_e[:], in0=h1m_es[:], in1=h1m_es[:],
                        op=mybir.AluOpType.mult,
                    )
                    nc.vector.tensor_scalar_mul(
                        ecor[:, 1:2], sq_e[:], float(n_empty)
                    )
                    nc.vector.tensor_tensor(
                        out=stat2[:], in0=stat2[:], in1=ecor[:],
                        op=mybir.AluOpType.subtract,
                    )
                nc.sync.dma_start(out=bn_in[:], in_=stat2[:])
                if no_collectives:
                    nc.sync.dma_start(out=bn_out[:], in_=bn_in[:])
                else:
                    nc.gpsimd.collective_compute(
                        "AllReduce",
                        mybir.AluOpType.add,
                        replica_groups=rg,
                        ins=[bn_in.ap().opt()],
                        outs=[bn_out.ap().opt()],
                    )
                sb_bn = vec_p.tile([P, 2], FP, tag="sb_bn")
                nc.sync.dma_start(out=sb_bn[:], in_=bn_out[:])

                mu = vec_p.tile([P, 1], FP, tag="mu")
                nc.vector.tensor_scalar_mul(mu[:], sb_bn[:, 0:1], 1.0 / n_bn)
                var = vec_p.tile([P, 1], FP, tag="var")
                nc.vector.tensor_scalar_mul(var[:], sb_bn[:, 1:2], 1.0 / n_bn)
                mu2 = vec_p.tile([P, 1], FP, tag="mu2")
                nc.vector.tensor_tensor(
                    out=mu2[:], in0=mu[:], in1=mu[:], op=mybir.AluOpType.mult
                )
                nc.vector.tensor_tensor(
                    out=var[:], in0=var[:], in1=mu2[:], op=mybir.AluOpType.subtract
                )
                sd = vec_p.tile([P, 1], FP, tag="sd")
                nc.scalar.activation(
                    out=sd[:], in_=var[:],
                    func=mybir.ActivationFunctionType.Sqrt, bias=sb_eps[:],
                )
                rinv = vec_p.tile([P, 1], FP, tag="rinv")
                nc.vector.reciprocal(rinv[:], sd[:])
                a_ap = vec_p.tile([P, 1], FP, tag="a")
                nc.vector.tensor_tensor(
                    out=a_ap[:], in0=rinv[:], in1=g_ap, op=mybir.AluOpType.mult
                )
                c_ap = vec_p.tile([P, 1], FP, tag="c")
                nc.vector.tensor_tensor(
                    out=c_ap[:], in0=mu[:], in1=a_ap[:], op=mybir.AluOpType.mult
                )
                nc.vector.tensor_tensor(
                    out=c_ap[:], in0=bt_ap, in1=c_ap[:], op=mybir.AluOpType.subtract
                )

                if li == 0:
                    # constant column of an empty slot after layer-1 phase 2,
                    # pre-scaled by (1+eps2): relu((1+eps2)*(w1b^T relu(c1))
                    # + (1+eps2)*b1b)
                    h1n_e = h0_p.tile([P, 1], BF, tag="h1n_e")
                    nc.scalar.activation(
                        out=h1n_e[:], in_=sb_zero[:],
                        func=mybir.ActivationFunctionType.Relu, bias=c_ap[:],
                    )
                    h2_et = mlpp.tile([P, W], FP, space="PSUM", tag="mlp")
                    h2_e = h2_et[:, 0:1]
                    nc.tensor.matmul(
                        h2_e, lhsT=wb[:], rhs=h1n_e[:], start=True, stop=True
                    )
                    nc.scalar.activation(
                        out=sb_own_e[:], in_=h2_e,
                        func=mybir.ActivationFunctionType.Relu,
                        scale=float(1.0 + eps2),
                        bias=sb_vecs[:, 6:7],
                    )

                # ---- phase 2 ----
                for wg in range(nwin):
                    h1n = h0_p.tile([P, W], BF, tag="h1n")
                    nc.scalar.activation(
                        out=h1n[:],
                        in_=sb_h1m[:, wg * W : wg * W + W],
                        func=mybir.ActivationFunctionType.Relu,
                        scale=a_ap[:],
                        bias=c_ap[:],
                    )
                    h2 = mlpp.tile([P, W], FP, space="PSUM", tag="mlp")
                    nc.tensor.matmul(
                        h2[:], lhsT=wb[:], rhs=h1n[:], start=True, stop=True
                    )
                    if li == 0:
                        nc.vector.tensor_scalar(
                            out=sb_hl1[:, wg * W : wg * W + W],
                            in0=h2[:],
                            scalar1=bb_ap,
                            scalar2=0.0,
                            op0=mybir.AluOpType.add,
                            op1=mybir.AluOpType.max,
                        )
                        trp_t = trp.tile([P, P], BF, space="PSUM", tag="trp")
                        nc.tensor.transpose(
                            out=trp_t[:],
                            in_=sb_hl1[:, wg * W : wg * W + W],
                            identity=sb_ident[:],
                        )
                        trs = trs_p.tile([P, P], BF, tag="trs")
                        nc.vector.tensor_scalar_add(trs[:], trp_t[:], 0.0)
                        nc.sync.dma_start(
                            out=h_shard[wg * P : (wg + 1) * P, :], in_=trs[:]
                        )
                    else:
                        of32 = sc_p.tile([P, W], FP, tag="of32")
                        nc.vector.tensor_scalar_add(of32[:], h2[:], bb_ap)
                        nc.sync.dma_start(
                            out=out_ext[:, wg * W : wg * W + W], in_=of32[:]
                        )

                if li == 0:
                    if no_collectives:
                        nc.sync.dma_start(
                            out=h_tab[core0 * spc : (core0 + 1) * spc, :],
                            in_=h_shard[:],
                        )
                    else:
                        nc.gpsimd.collective_compute(
                            "AllGather",
                            mybir.AluOpType.bypass,
                            replica_groups=rg,
                            ins=[h_shard.ap().opt()],
                            outs=[h_tab.ap().opt()],
                        )

            layer(0)
            layer(1)

    nc.compile()
    return nc


# ----------------------------------------------------------------------------
# entry
# ----------------------------------------------------------------------------

def _make_inputs(pr, inputs, eps2):
    vecs = np.stack(
        [
            np.asarray(inputs["g1"], np.float32),
            np.asarray(inputs["bt1"], np.float32),
            np.asarray(inputs["b1b"], np.float32),
            np.asarray(inputs["g2"], np.float32),
            np.asarray(inputs["bt2"], np.float32),
            np.asarray(inputs["b2b"], np.float32),
            (1.0 + eps2) * np.asarray(inputs["b1b"], np.float32),
        ],
        axis=1,
    )
    bfl = ml_dtypes.bfloat16
    iota = np.tile(np.arange(W, dtype=np.float32), (P, 1)).astype(bfl)
    ident = np.eye(P, dtype=np.float32).astype(bfl)
    w = {
        k: np.asarray(inputs[k], np.float32).astype(bfl)
        for k in ("w1a", "w1b", "w2a", "w2b")
    }
    in_maps = []
    for c in range(NCORES):
        in_maps.append(
            dict(
                x_tab=pr["x_tab"],
                idxs=pr["idx_sb"][c],
                dlocs=pr["dloc_sb"][c],
                x_own=pr["x_own"][c],
                vecs=vecs, iota=iota, ident=ident, **w,
            )
        )
    return in_maps


def _run(inputs, tpc, n_bn, trace=False):
    x = np.asarray(inputs["x"], np.float32)
    src = np.asarray(inputs["src"], np.int64)
    dst = np.asarray(inputs["dst"], np.int64)
    eps1 = float(np.asarray(inputs["eps1"]))
    eps2 = float(np.asarray(inputs["eps2"]))

    pr = _prep(x, src, dst, eps1, tpc)
    nc = _build(tpc, pr, eps2, n_bn)
    in_maps = _make_inputs(pr, inputs, eps2)
    res = bass_utils.run_bass_kernel_spmd(
        nc, in_maps, list(range(NCORES)), trace=trace
    )
    outs = [np.asarray(res.results[c]["out"], np.float32).T for c in range(NCORES)]
    out_slot = np.concatenate(outs, axis=0)
    nos = pr["node_of_slot"]
    m = nos >= 0
    out = np.zeros((x.shape[0], F), np.float32)
    out[nos[m]] = out_slot[m]
    if trace:
        return out, res
    return out


def kernel(**inputs) -> np.ndarray:
    return _run(inputs, TPC_FULL, N_FULL)
